# revision 4
# baseline (speedup 1.0000x reference)
"""Trainium2 Bass kernel for nn_BaseCompressor2 (truncated one-pole IIR
compressor), v3: time-on-partitions layout, scan as a matmul FIR.

Layout B: per (batch, channel) the host pre-transposes the signal so that
SBUF partition i holds samples t with t mod 128 == i, free axis u = t div
128 (U = 2048 columns per batch).  All per-batch params are replicated
across the 128 partitions, so every elementwise op sees per-partition
scalars as usual.

The one-pole IIR (effective FIR window <= 128*M samples since alpha^k
underflows) becomes M matmul taps on the otherwise idle PE:
  y[:, piece] = sum_m H_m @ E_pad[:, piece - 128 m]
with H_m[k, po] = alpha^(128 m + po - k) (H_0 lower-triangular), E zero-
padded on the left, accumulating f32 in PSUM.  No scan, no carries.

Engine split per batch (U=2048 cols):
  Act:  sq = Square(s) (both ch), x = Ln(escale*y+eps) from PSUM,
        g = Exp(-h)
  DVE:  E = sq0+sq1, u=(x+uk)max0, v=(u min knee)*rta, d=(x+ukk)max0,
        t=v*v, h=(d*negc1)+t, out_c = g*s_c   (all fp16 2x/4x modes)
  PE :  M*4 matmuls of [128,128]x[128,512] fp16 -> PSUM
  DMA:  1 fat input DMA + 1 fat output DMA per batch (fp16 both ways)

Host does the (free) layout transposes and the final f32 cast.
"""

import numpy as np

N, C, L = 32, 2, 262144
NCORES = 8
BPC = N // NCORES          # batches per core
P = 128
U = L // P                 # 2048 free columns per batch
PIECE = 512                # psum bank width (f32)
NPIECE = U // PIECE
NP = 6                     # param columns per batch
ESCALE, UK, UKK, KNEE, RTA, NEGC1 = range(NP)

_cache = {}


def _host_params(z_alpha_pre, log_threshold, log_ratio, log_knee):
    z = z_alpha_pre.astype(np.float64).reshape(-1)
    thr = log_threshold.astype(np.float64).reshape(-1) - 6.0
    knee = np.exp(log_knee.astype(np.float64).reshape(-1))
    r001 = 1.0 + np.exp(log_ratio.astype(np.float64).reshape(-1)) + 0.001
    alpha = 1.0 / (1.0 + np.exp(-z))
    negc1 = 1.0 - 1.0 / r001
    vals = np.zeros((N, NP), dtype=np.float64)
    vals[:, ESCALE] = (1.0 - alpha) / 2.0
    vals[:, UK] = knee / 2.0 - thr
    vals[:, UKK] = -knee / 2.0 - thr
    vals[:, KNEE] = knee
    vals[:, RTA] = np.sqrt(negc1 / (2.0 * (knee + 0.001)))
    vals[:, NEGC1] = negc1
    # taps needed per batch: alpha^k < 6e-8 (fp16 subnormal floor) cut
    lna = np.log(alpha)
    kmax = np.ceil(16.7 / np.maximum(1e-9, -lna)).astype(np.int64)
    m_b = (kmax + 127) // 128 + 1
    M = int(min(16, max(m_b)))
    return vals.astype(np.float32), alpha, M


def _host_weights(alpha, M):
    """FIR tap matrices per batch: H[b, m][k, po] = a^(128m+po-k), masked."""
    po = np.arange(P)[None, :]
    k = np.arange(P)[:, None]
    out = np.zeros((N, M, P, P), dtype=np.float16)
    for n in range(N):
        lna = np.log(alpha[n])
        for m in range(M):
            e = (128 * m + po - k).astype(np.float64)
            h = np.exp(e * lna)
            h[e < 0] = 0.0
            h[h < 6e-8] = 0.0
            out[n, m] = h.astype(np.float16)
    return out


def _build_program(M):
    from contextlib import ExitStack

    import concourse.bacc as bacc
    import concourse.bass as bass
    import concourse.tile as tile
    from concourse import mybir

    dt = mybir.dt.float32
    dh = mybir.dt.float16
    Alu = mybir.AluOpType
    Af = mybir.ActivationFunctionType

    PAD = M - 1
    nc = bacc.Bacc(
        "TRN2", target_bir_lowering=False, debug=False,
        enable_asserts=False, num_devices=NCORES,
    )
    sigB = nc.dram_tensor("sigB", [C, P, BPC * U], dh, kind="ExternalInput")
    pcols = nc.dram_tensor("pcols", [P, BPC * NP], dt, kind="ExternalInput")
    wts = nc.dram_tensor("wts", [P, BPC * M * P], dh, kind="ExternalInput")
    out = nc.dram_tensor("out", [BPC, C, P, U], dh, kind="ExternalOutput")

    with tile.TileContext(nc) as tc, ExitStack() as ctx:
        const = ctx.enter_context(tc.tile_pool(name="const", bufs=1))
        spool = ctx.enter_context(tc.tile_pool(name="sp", bufs=1))
        sqp = ctx.enter_context(tc.tile_pool(name="sq", bufs=2))
        epool = ctx.enter_context(tc.tile_pool(name="ep", bufs=2))
        wkp = ctx.enter_context(tc.tile_pool(name="wk", bufs=3))
        opool = ctx.enter_context(tc.tile_pool(name="op", bufs=2))
        psy = ctx.enter_context(tc.tile_pool(name="psy", bufs=2, space="PSUM"))

        pc = const.tile([P, BPC * NP], dt, tag="pc")
        wt = const.tile([P, BPC * M * P], dh, tag="wt")
        epsc = const.tile([P, 1], dt, tag="epsc")

        def col(b, j):
            return pc[:, b * NP + j:b * NP + j + 1]

        # manual activation-table load: natural_log_exp_and_others (id 6)
        ld = mybir.InstLoadActFuncSet(
            name=nc.get_next_instruction_name(), act_func_set_id=6,
            ins=[], outs=[])
        ld.engine = mybir.EngineType.Activation
        nc.scalar.add_instruction(ld)
        nc.vector.memset(epsc, 1e-5)

        # ---- input DMAs up front ----
        s = [spool.tile([P, C, U], dh, tag=f"s{b}", name=f"s{b}")
             for b in range(BPC)]
        for b in range(BPC):
            nc.sync.dma_start(
                s[b],
                bass.AP(sigB, b * U,
                        [[BPC * U, P], [P * BPC * U, C], [1, U]]))
        nc.sync.dma_start(pc, pcols.ap())
        nc.sync.dma_start(wt, wts.ap())

        st = [dict() for _ in range(BPC)]

        def p_sq(b):
            sq = sqp.tile([P, C, U], dh, tag="sq", name=f"sq{b}")
            nc.scalar.activation(sq, s[b], Af.Square)
            st[b]["sq"] = sq

        def p_ee(b):
            sq = st[b]["sq"]
            ep = epool.tile([P, PAD + U], dh, tag="E", name=f"E{b}")
            if PAD:
                nc.gpsimd.memset(ep[:, 0:PAD], 0.0)
            nc.vector.tensor_tensor(ep[:, PAD:PAD + U], sq[:, 0, :],
                                    sq[:, 1, :], Alu.add)
            st[b]["E"] = ep

        def p_mm(b):
            ep = st[b]["E"]
            y = psy.tile([P, U], dt, tag="y", name=f"y{b}")
            for m in range(M):
                w = wt[:, (b * M + m) * P:(b * M + m + 1) * P]
                for q in range(NPIECE):
                    c0 = PAD + PIECE * q - m
                    nc.tensor.matmul(
                        y[:, PIECE * q:PIECE * (q + 1)],
                        w, ep[:, c0:c0 + PIECE],
                        start=(m == 0), stop=(m == M - 1))
            st[b]["y"] = y

        def p_ln(b):
            x = wkp.tile([P, U], dh, tag="x", name=f"x{b}")
            nc.scalar.activation(x, st[b]["y"], Af.Ln,
                                 scale=col(b, ESCALE), bias=epsc[:, 0:1])
            st[b]["x"] = x

        def p_uvd(b):
            x = st[b]["x"]
            uv = wkp.tile([P, U], dh, tag="uv", name=f"uv{b}")
            dd = wkp.tile([P, U], dh, tag="d", name=f"d{b}")
            nc.vector.tensor_scalar(uv, x, col(b, UK), 0.0, Alu.add, Alu.max)
            nc.vector.tensor_scalar(dd, x, col(b, UKK), 0.0, Alu.add, Alu.max)
            nc.vector.tensor_scalar(uv, uv, col(b, KNEE), col(b, RTA),
                                    Alu.min, Alu.mult)
            st[b]["v"], st[b]["d"] = uv, dd

        def p_th(b):
            v, d = st[b]["v"], st[b]["d"]
            # t = v*v overwrites x (dead); h = negc1*d + t overwrites d
            t = st[b]["x"]
            nc.vector.tensor_tensor(t, v, v, Alu.mult)
            nc.vector.scalar_tensor_tensor(d, d, col(b, NEGC1), t,
                                           Alu.mult, Alu.add)
            st[b]["h"] = d

        def p_exp(b):
            g = st[b]["v"]  # overwrite v (dead)
            nc.scalar.activation(g, st[b]["h"], Af.Exp, scale=-1.0)
            st[b]["g"] = g

        def p_mul(b):
            g = st[b]["g"]
            o = opool.tile([P, C, U], dh, tag="o", name=f"o{b}")
            nc.vector.tensor_tensor(o[:, 0, :], g, s[b][:, 0, :], Alu.mult)
            nc.vector.tensor_tensor(o[:, 1, :], g, s[b][:, 1, :], Alu.mult)
            nc.gpsimd.dma_start(
                bass.AP(out, b * C * P * U,
                        [[U, P], [P * U, C], [1, U]]), o)

        # ---- interleaved emission for pipeline flow ----
        p_sq(0); p_ee(0); p_mm(0)
        p_sq(1); p_ee(1)
        p_ln(0); p_uvd(0); p_th(0)
        p_mm(1)
        p_sq(2); p_ee(2)
        p_exp(0); p_ln(1)
        p_mul(0)
        p_uvd(1); p_th(1)
        p_mm(2)
        p_sq(3); p_ee(3)
        p_exp(1); p_ln(2)
        p_mul(1)
        p_uvd(2); p_th(2)
        p_mm(3)
        p_exp(2); p_ln(3)
        p_mul(2)
        p_uvd(3); p_th(3)
        p_exp(3)
        p_mul(3)

    nc.compile()
    return nc


def _get_program(M):
    key = ("nc", M)
    if key not in _cache:
        _cache[key] = _build_program(M)
    return _cache[key]


def _run(inputs, trace=False):
    from concourse.bass_utils import run_bass_kernel_spmd

    sig16 = np.asarray(inputs["input_signals"], np.float32).astype(np.float16)
    # layout B: [N, C, P, U] with [n,c,p,u] = sig[n,c,128*u+p]
    sB = np.ascontiguousarray(sig16.reshape(N, C, U, P).swapaxes(2, 3))
    pv, alpha, M = _host_params(
        np.asarray(inputs["z_alpha_pre"], np.float32),
        np.asarray(inputs["log_threshold"], np.float32),
        np.asarray(inputs["log_ratio"], np.float32),
        np.asarray(inputs["log_knee"], np.float32),
    )
    wts_all = _host_weights(alpha, M)
    nc = _get_program(M)

    in_maps = []
    for cid in range(NCORES):
        bsl = slice(cid * BPC, (cid + 1) * BPC)
        # sigB [C, P, BPC*U]
        core_sig = np.ascontiguousarray(
            sB[bsl].transpose(1, 2, 0, 3).reshape(C, P, BPC * U))
        # params replicated across partitions: [P, BPC*NP]
        cols = np.ascontiguousarray(
            np.tile(pv[bsl].reshape(1, BPC * NP), (P, 1)))
        # weights [P, BPC*M*P]: wt[k, (b*M+m)*P+po] = H[b,m,k,po]
        wcore = np.ascontiguousarray(
            wts_all[bsl].transpose(2, 0, 1, 3).reshape(P, BPC * M * P))
        in_maps.append({"sigB": core_sig, "pcols": cols, "wts": wcore})

    res = run_bass_kernel_spmd(
        nc, in_maps, core_ids=list(range(NCORES)), trace=trace,
    )
    outp = np.empty((N, C, L), dtype=np.float32)
    for cid in range(NCORES):
        o = res.results[cid]["out"]  # [BPC, C, P, U] fp16
        outp[cid * BPC:(cid + 1) * BPC] = (
            o.transpose(0, 1, 3, 2).reshape(BPC, C, L).astype(np.float32))
    return outp, res


def kernel(**inputs) -> np.ndarray:
    out, _ = _run(inputs, trace=False)
    return out


# revision 5
# speedup vs baseline: 1.0173x; 1.0173x over previous
"""Trainium2 Bass kernel for nn_BaseCompressor2 (truncated one-pole IIR
compressor), v4: time-on-partitions layout, scan as a matmul FIR.

Layout B: per (batch, channel) the host pre-transposes the signal so that
SBUF partition i holds samples t with t mod 128 == i, free axis u = t div
128 (U = 2048 columns per batch).  All per-batch params are replicated
across the 128 partitions, so every elementwise op sees per-partition
scalars as usual.

The one-pole IIR (effective FIR window <= 128*M samples since alpha^k
underflows) becomes M matmul taps on the otherwise idle PE:
  y[:, j] = sum_m H_m @ E[:, j - m]        (f32 accum in PSUM)
with H_m[k, po] = alpha^(128 m + po - k) (H_0 lower-triangular).  Edge
columns j < m simply skip the tap (causal zero history).  No scan, no
carries, no cross-partition traffic.

Engine split per batch (U=2048 cols, all fp16):
  Act:  sq = Square(s) (both ch), x = Ln(escale*y+eps) from PSUM,
        g = Exp(-h) from PSUM
  DVE:  E = sq0+sq1, u=(x+uk)max0, v=(u min knee)*rta, d=(x+ukk)max0,
        t=v*v, out = g*s (channel-fused via stride-0 broadcast of g)
  PE :  M*4 FIR matmuls + h = eye@t + (negc1*eye)@d into PSUM
  DMA:  1 fat input DMA + 1 fat output DMA per batch (fp16 both ways)

Host does the (free) layout transposes and the final f32 cast.
"""

import numpy as np

N, C, L = 32, 2, 262144
NCORES = 8
BPC = N // NCORES          # batches per core
P = 128
U = L // P                 # 2048 free columns per batch
PIECE = 512                # psum bank width (f32)
NPIECE = U // PIECE
NP = 6                     # param columns per batch
ESCALE, UK, UKK, KNEE, RTA, NEGC1 = range(NP)

_cache = {}


def _host_params(z_alpha_pre, log_threshold, log_ratio, log_knee):
    z = z_alpha_pre.astype(np.float64).reshape(-1)
    thr = log_threshold.astype(np.float64).reshape(-1) - 6.0
    knee = np.exp(log_knee.astype(np.float64).reshape(-1))
    r001 = 1.0 + np.exp(log_ratio.astype(np.float64).reshape(-1)) + 0.001
    alpha = 1.0 / (1.0 + np.exp(-z))
    negc1 = 1.0 - 1.0 / r001
    vals = np.zeros((N, NP), dtype=np.float64)
    vals[:, ESCALE] = (1.0 - alpha) / 2.0
    vals[:, UK] = knee / 2.0 - thr
    vals[:, UKK] = -knee / 2.0 - thr
    vals[:, KNEE] = knee
    vals[:, RTA] = np.sqrt(negc1 / (2.0 * (knee + 0.001)))
    vals[:, NEGC1] = negc1
    # taps needed per batch: alpha^k < 6e-8 (fp16 subnormal floor) cut
    lna = np.log(alpha)
    kmax = np.ceil(16.7 / np.maximum(1e-9, -lna)).astype(np.int64)
    m_b = (kmax + 127) // 128 + 1
    M = int(min(16, max(m_b)))
    return vals.astype(np.float32), alpha, negc1, M


def _host_weights(alpha, M):
    """FIR tap matrices per batch: H[b, m][k, po] = a^(128m+po-k), masked."""
    po = np.arange(P)[None, :]
    k = np.arange(P)[:, None]
    out = np.zeros((N, M, P, P), dtype=np.float16)
    for n in range(N):
        lna = np.log(alpha[n])
        for m in range(M):
            e = (128 * m + po - k).astype(np.float64)
            h = np.exp(e * lna)
            h[e < 0] = 0.0
            h[h < 6e-8] = 0.0
            out[n, m] = h.astype(np.float16)
    return out


def _build_program(M):
    from contextlib import ExitStack

    import concourse.bacc as bacc
    import concourse.bass as bass
    import concourse.tile as tile
    from concourse import mybir

    dt = mybir.dt.float32
    dh = mybir.dt.float16
    Alu = mybir.AluOpType
    Af = mybir.ActivationFunctionType

    nc = bacc.Bacc(
        "TRN2", target_bir_lowering=False, debug=False,
        enable_asserts=False, num_devices=NCORES,
    )
    sigB = nc.dram_tensor("sigB", [C, P, BPC * U], dh, kind="ExternalInput")
    pcols = nc.dram_tensor("pcols", [P, BPC * NP], dt, kind="ExternalInput")
    wts = nc.dram_tensor("wts", [P, BPC * M * P], dh, kind="ExternalInput")
    # diag weights for h: cols [b*P:(b+1)*P] = negc1_b * I, then I
    wts2 = nc.dram_tensor("wts2", [P, (BPC + 1) * P], dh,
                          kind="ExternalInput")
    out = nc.dram_tensor("out", [BPC, C, P, U], dh, kind="ExternalOutput")

    with tile.TileContext(nc) as tc, ExitStack() as ctx:
        const = ctx.enter_context(tc.tile_pool(name="const", bufs=1))
        spool = ctx.enter_context(tc.tile_pool(name="sp", bufs=1))
        sqp = ctx.enter_context(tc.tile_pool(name="sq", bufs=2))
        epool = ctx.enter_context(tc.tile_pool(name="ep", bufs=2))
        wkp = ctx.enter_context(tc.tile_pool(name="wk", bufs=3))
        opool = ctx.enter_context(tc.tile_pool(name="op", bufs=2))
        psy = ctx.enter_context(tc.tile_pool(name="psy", bufs=1, space="PSUM"))
        psh = ctx.enter_context(tc.tile_pool(name="psh", bufs=1, space="PSUM"))

        pc = const.tile([P, BPC * NP], dt, tag="pc")
        wt = const.tile([P, BPC * M * P], dh, tag="wt")
        wt2 = const.tile([P, (BPC + 1) * P], dh, tag="wt2")
        epsc = const.tile([P, 1], dt, tag="epsc")

        def col(b, j):
            return pc[:, b * NP + j:b * NP + j + 1]

        # manual activation-table load: natural_log_exp_and_others (id 6)
        ld = mybir.InstLoadActFuncSet(
            name=nc.get_next_instruction_name(), act_func_set_id=6,
            ins=[], outs=[])
        ld.engine = mybir.EngineType.Activation
        nc.scalar.add_instruction(ld)
        nc.vector.memset(epsc, 1e-5)

        # ---- input DMAs up front ----
        s = [spool.tile([P, C, U], dh, tag=f"s{b}", name=f"s{b}")
             for b in range(BPC)]
        for b in range(BPC):
            nc.sync.dma_start(
                s[b],
                bass.AP(sigB, b * U,
                        [[BPC * U, P], [P * BPC * U, C], [1, U]]))
        nc.sync.dma_start(pc, pcols.ap())
        nc.sync.dma_start(wt, wts.ap())
        nc.sync.dma_start(wt2, wts2.ap())

        st = [dict() for _ in range(BPC)]

        def p_sq(b):
            sq = sqp.tile([P, C, U], dh, tag="sq", name=f"sq{b}")
            nc.scalar.activation(sq, s[b], Af.Square)
            st[b]["sq"] = sq

        def p_ee(b):
            sq = st[b]["sq"]
            ep = epool.tile([P, U], dh, tag="E", name=f"E{b}")
            nc.vector.tensor_tensor(ep, sq[:, 0, :], sq[:, 1, :], Alu.add)
            st[b]["E"] = ep

        def p_mm(b):
            ep = st[b]["E"]
            y = psy.tile([P, U], dt, tag="y", name=f"y{b}")
            for m in range(M):
                w = wt[:, (b * M + m) * P:(b * M + m + 1) * P]
                for q in range(NPIECE):
                    lo = PIECE * q
                    off = m if q == 0 else 0
                    nc.tensor.matmul(
                        y[:, lo + off:lo + PIECE],
                        w, ep[:, lo + off - m:lo + PIECE - m],
                        start=(m == 0), stop=(m == M - 1),
                        skip_group_check=True)
            st[b]["y"] = y

        def p_ln(b):
            x = wkp.tile([P, U], dh, tag="x", name=f"x{b}")
            nc.scalar.activation(x, st[b]["y"], Af.Ln,
                                 scale=col(b, ESCALE), bias=epsc[:, 0:1])
            st[b]["x"] = x

        def p_uvd(b):
            x = st[b]["x"]
            uv = wkp.tile([P, U], dh, tag="uv", name=f"uv{b}")
            dd = wkp.tile([P, U], dh, tag="d", name=f"d{b}")
            nc.vector.tensor_scalar(uv, x, col(b, UK), 0.0, Alu.add, Alu.max)
            nc.vector.tensor_scalar(dd, x, col(b, UKK), 0.0, Alu.add, Alu.max)
            nc.vector.tensor_scalar(uv, uv, col(b, KNEE), col(b, RTA),
                                    Alu.min, Alu.mult)
            st[b]["v"], st[b]["d"] = uv, dd

        def p_t(b):
            t = st[b]["x"]  # overwrite x (dead)
            nc.vector.tensor_tensor(t, st[b]["v"], st[b]["v"], Alu.mult)
            st[b]["t"] = t

        def p_h(b):
            t, d = st[b]["t"], st[b]["d"]
            h = psh.tile([P, U], dt, tag="h", name=f"h{b}")
            eye = wt2[:, BPC * P:(BPC + 1) * P]
            nI = wt2[:, b * P:(b + 1) * P]
            for q in range(NPIECE):
                lo = PIECE * q
                nc.tensor.matmul(h[:, lo:lo + PIECE], eye, t[:, lo:lo + PIECE],
                                 start=True, stop=False)
            for q in range(NPIECE):
                lo = PIECE * q
                nc.tensor.matmul(h[:, lo:lo + PIECE], nI, d[:, lo:lo + PIECE],
                                 start=False, stop=True)
            st[b]["h"] = h

        def p_exp(b):
            g = st[b]["v"]  # overwrite v (dead)
            nc.scalar.activation(g, st[b]["h"], Af.Exp, scale=-1.0)
            st[b]["g"] = g

        def p_mul(b):
            g = st[b]["g"]
            o = opool.tile([P, C, U], dh, tag="o", name=f"o{b}")
            grep = bass.AP(g.tensor, g.offset,
                           [[g.ap[0][0], P], [0, C], [1, U]])
            nc.vector.tensor_tensor(o, grep, s[b], Alu.mult)
            nc.gpsimd.dma_start(
                bass.AP(out, b * C * P * U,
                        [[U, P], [P * U, C], [1, U]]), o)

        # ---- interleaved emission for pipeline flow ----
        p_sq(0); p_ee(0); p_mm(0)
        p_sq(1); p_ee(1)
        p_ln(0); p_uvd(0); p_t(0); p_h(0)
        p_mm(1)
        p_sq(2); p_ee(2)
        p_exp(0); p_ln(1)
        p_mul(0)
        p_uvd(1); p_t(1); p_h(1)
        p_mm(2)
        p_sq(3); p_ee(3)
        p_exp(1); p_ln(2)
        p_mul(1)
        p_uvd(2); p_t(2); p_h(2)
        p_mm(3)
        p_exp(2); p_ln(3)
        p_mul(2)
        p_uvd(3); p_t(3); p_h(3)
        p_exp(3)
        p_mul(3)

    nc.compile()
    return nc


def _get_program(M):
    key = ("nc", M)
    if key not in _cache:
        _cache[key] = _build_program(M)
    return _cache[key]


def _run(inputs, trace=False):
    from concourse.bass_utils import run_bass_kernel_spmd

    sig16 = np.asarray(inputs["input_signals"], np.float32).astype(np.float16)
    # layout B: [N, C, P, U] with [n,c,p,u] = sig[n,c,128*u+p]
    sB = np.ascontiguousarray(sig16.reshape(N, C, U, P).swapaxes(2, 3))
    pv, alpha, negc1, M = _host_params(
        np.asarray(inputs["z_alpha_pre"], np.float32),
        np.asarray(inputs["log_threshold"], np.float32),
        np.asarray(inputs["log_ratio"], np.float32),
        np.asarray(inputs["log_knee"], np.float32),
    )
    wts_all = _host_weights(alpha, M)
    nc = _get_program(M)

    eye = np.eye(P, dtype=np.float16)
    in_maps = []
    for cid in range(NCORES):
        bsl = slice(cid * BPC, (cid + 1) * BPC)
        core_sig = np.ascontiguousarray(
            sB[bsl].transpose(1, 2, 0, 3).reshape(C, P, BPC * U))
        cols = np.ascontiguousarray(
            np.tile(pv[bsl].reshape(1, BPC * NP), (P, 1)))
        wcore = np.ascontiguousarray(
            wts_all[bsl].transpose(2, 0, 1, 3).reshape(P, BPC * M * P))
        w2 = np.concatenate(
            [np.concatenate([eye * np.float16(negc1[n]) for n in
                             range(cid * BPC, (cid + 1) * BPC)], axis=1),
             eye], axis=1)
        in_maps.append({"sigB": core_sig, "pcols": cols, "wts": wcore,
                        "wts2": np.ascontiguousarray(w2)})

    res = run_bass_kernel_spmd(
        nc, in_maps, core_ids=list(range(NCORES)), trace=trace,
    )
    outp = np.empty((N, C, L), dtype=np.float32)
    for cid in range(NCORES):
        o = res.results[cid]["out"]  # [BPC, C, P, U] fp16
        outp[cid * BPC:(cid + 1) * BPC] = (
            o.transpose(0, 1, 3, 2).reshape(BPC, C, L).astype(np.float32))
    return outp, res


def kernel(**inputs) -> np.ndarray:
    out, _ = _run(inputs, trace=False)
    return out


# revision 7
# speedup vs baseline: 1.0841x; 1.0657x over previous
"""Trainium2 Bass kernel for nn_BaseCompressor2 (truncated one-pole IIR
compressor), v4: time-on-partitions layout, scan as a matmul FIR.

Layout B: per (batch, channel) the host pre-transposes the signal so that
SBUF partition i holds samples t with t mod 128 == i, free axis u = t div
128 (U = 2048 columns per batch).  All per-batch params are replicated
across the 128 partitions, so every elementwise op sees per-partition
scalars as usual.

The one-pole IIR (effective FIR window <= 128*M samples since alpha^k
underflows) becomes M matmul taps on the otherwise idle PE:
  y[:, j] = sum_m H_m @ E[:, j - m]        (f32 accum in PSUM)
with H_m[k, po] = alpha^(128 m + po - k) (H_0 lower-triangular).  Edge
columns j < m simply skip the tap (causal zero history).  No scan, no
carries, no cross-partition traffic.

Engine split per batch (U=2048 cols, all fp16):
  Act:  sq = Square(s) (both ch), x = Ln(escale*y+eps) from PSUM,
        g = Exp(-h) from PSUM
  DVE:  E = sq0+sq1, u=(x+uk)max0, v=(u min knee)*rta, d=(x+ukk)max0,
        t=v*v, out = g*s (channel-fused via stride-0 broadcast of g)
  PE :  M*4 FIR matmuls + h = eye@t + (negc1*eye)@d into PSUM
  DMA:  1 fat input DMA + 1 fat output DMA per batch (fp16 both ways)

Host does the (free) layout transposes and the final f32 cast.
"""

import numpy as np

N, C, L = 32, 2, 262144
NCORES = 8
BPC = N // NCORES          # batches per core
P = 128
U = L // P                 # 2048 free columns per batch
PIECE = 512                # psum bank width (f32)
NPIECE = U // PIECE
NP = 6                     # param columns per batch
ESCALE, UK, UKK, KNEE, RTA, NEGC1 = range(NP)

_cache = {}


def _host_params(z_alpha_pre, log_threshold, log_ratio, log_knee):
    z = z_alpha_pre.astype(np.float64).reshape(-1)
    thr = log_threshold.astype(np.float64).reshape(-1) - 6.0
    knee = np.exp(log_knee.astype(np.float64).reshape(-1))
    r001 = 1.0 + np.exp(log_ratio.astype(np.float64).reshape(-1)) + 0.001
    alpha = 1.0 / (1.0 + np.exp(-z))
    negc1 = 1.0 - 1.0 / r001
    vals = np.zeros((N, NP), dtype=np.float64)
    vals[:, ESCALE] = (1.0 - alpha) / 2.0
    vals[:, UK] = knee / 2.0 - thr
    vals[:, UKK] = -knee / 2.0 - thr
    vals[:, KNEE] = knee
    vals[:, RTA] = np.sqrt(negc1 / (2.0 * (knee + 0.001)))
    vals[:, NEGC1] = negc1
    # taps needed per batch: alpha^k < 6e-8 (fp16 subnormal floor) cut
    lna = np.log(alpha)
    kmax = np.ceil(16.7 / np.maximum(1e-9, -lna)).astype(np.int64)
    m_b = (kmax + 127) // 128 + 1
    M = int(min(16, max(m_b)))
    return vals.astype(np.float32), alpha, negc1, M


def _host_weights(alpha, M):
    """FIR tap matrices per batch: H[b, m][k, po] = a^(128m+po-k), masked."""
    po = np.arange(P)[None, :]
    k = np.arange(P)[:, None]
    out = np.zeros((N, M, P, P), dtype=np.float16)
    for n in range(N):
        lna = np.log(alpha[n])
        for m in range(M):
            e = (128 * m + po - k).astype(np.float64)
            h = np.exp(e * lna)
            h[e < 0] = 0.0
            h[h < 6e-8] = 0.0
            out[n, m] = h.astype(np.float16)
    return out


def _build_program(M):
    from contextlib import ExitStack

    import concourse.bacc as bacc
    import concourse.bass as bass
    import concourse.tile as tile
    from concourse import mybir

    dt = mybir.dt.float32
    dh = mybir.dt.float16
    Alu = mybir.AluOpType
    Af = mybir.ActivationFunctionType

    nc = bacc.Bacc(
        "TRN2", target_bir_lowering=False, debug=False,
        enable_asserts=False, num_devices=NCORES,
    )
    sigB = nc.dram_tensor("sigB", [C, P, BPC * U], dh, kind="ExternalInput")
    pcols = nc.dram_tensor("pcols", [P, BPC * NP], dt, kind="ExternalInput")
    wts = nc.dram_tensor("wts", [P, BPC * M * P], dh, kind="ExternalInput")
    # diag weights for h: cols [b*P:(b+1)*P] = negc1_b * I, then I
    wts2 = nc.dram_tensor("wts2", [P, (BPC + 1) * P], dh,
                          kind="ExternalInput")
    out = nc.dram_tensor("out", [BPC, C, P, U], dh, kind="ExternalOutput")

    H = U // 2             # half-batch columns

    with tile.TileContext(nc) as tc, ExitStack() as ctx:
        const = ctx.enter_context(tc.tile_pool(name="const", bufs=1))
        spool = ctx.enter_context(tc.tile_pool(name="sp", bufs=1))
        sqp = ctx.enter_context(tc.tile_pool(name="sq", bufs=3))
        epool = ctx.enter_context(tc.tile_pool(name="ep", bufs=2))
        wkp = ctx.enter_context(tc.tile_pool(name="wk", bufs=5))
        opool = ctx.enter_context(tc.tile_pool(name="op", bufs=3))
        psy = ctx.enter_context(tc.tile_pool(name="psy", bufs=2, space="PSUM"))
        psh = ctx.enter_context(tc.tile_pool(name="psh", bufs=2, space="PSUM"))

        pc = const.tile([P, BPC * NP], dt, tag="pc")
        wt = const.tile([P, BPC * M * P], dh, tag="wt")
        wt2 = const.tile([P, (BPC + 1) * P], dh, tag="wt2")
        epsc = const.tile([P, 1], dt, tag="epsc")

        def col(b, j):
            return pc[:, b * NP + j:b * NP + j + 1]

        # manual activation-table load: natural_log_exp_and_others (id 6)
        ld = mybir.InstLoadActFuncSet(
            name=nc.get_next_instruction_name(), act_func_set_id=6,
            ins=[], outs=[])
        ld.engine = mybir.EngineType.Activation
        nc.scalar.add_instruction(ld)
        nc.vector.memset(epsc, 1e-5)

        # ---- tiles / state ----
        s = [spool.tile([P, C, U], dh, tag=f"s{b}", name=f"s{b}")
             for b in range(BPC)]
        E = [None] * BPC
        NU = 2 * BPC       # half-batch pipeline units
        st = [dict() for _ in range(NU)]

        def p_in(i):
            b, k = divmod(i, 2)
            nc.sync.dma_start(
                s[b][:, :, k * H:(k + 1) * H],
                bass.AP(sigB, b * U + k * H,
                        [[BPC * U, P], [P * BPC * U, C], [1, H]]))

        def p_sq(i):
            b, k = divmod(i, 2)
            sq = sqp.tile([P, C, H], dh, tag="sq", name=f"sq{i}")
            nc.scalar.activation(sq, s[b][:, :, k * H:(k + 1) * H], Af.Square)
            st[i]["sq"] = sq

        def p_ee(i):
            b, k = divmod(i, 2)
            sq = st[i]["sq"]
            if k == 0:
                E[b] = epool.tile([P, U], dh, tag="E", name=f"E{b}")
            nc.vector.tensor_tensor(E[b][:, k * H:(k + 1) * H],
                                    sq[:, 0, :], sq[:, 1, :], Alu.add)

        def p_mm(i):
            b, k = divmod(i, 2)
            ep = E[b]
            y = psy.tile([P, H], dt, tag="y", name=f"y{i}")
            for m in range(M):
                w = wt[:, (b * M + m) * P:(b * M + m + 1) * P]
                for qq in range(2):
                    q = 2 * k + qq
                    lo = PIECE * q
                    off = m if q == 0 else 0
                    nc.tensor.matmul(
                        y[:, PIECE * qq + off:PIECE * (qq + 1)],
                        w, ep[:, lo + off - m:lo + PIECE - m],
                        start=(m == 0), stop=(m == M - 1),
                        skip_group_check=True)
            st[i]["y"] = y

        def p_ln(i):
            b, k = divmod(i, 2)
            x = wkp.tile([P, H], dh, tag="x", name=f"x{i}")
            nc.scalar.activation(x, st[i]["y"], Af.Ln,
                                 scale=col(b, ESCALE), bias=epsc[:, 0:1])
            st[i]["x"] = x

        def p_uvdt(i):
            b, k = divmod(i, 2)
            x = st[i]["x"]
            uv = wkp.tile([P, H], dh, tag="uv", name=f"uv{i}")
            dd = wkp.tile([P, H], dh, tag="d", name=f"d{i}")
            nc.vector.tensor_scalar(uv, x, col(b, UK), 0.0, Alu.add, Alu.max)
            nc.vector.tensor_scalar(dd, x, col(b, UKK), 0.0, Alu.add, Alu.max)
            nc.vector.tensor_scalar(uv, uv, col(b, KNEE), col(b, RTA),
                                    Alu.min, Alu.mult)
            t = x  # overwrite x (dead)
            nc.vector.tensor_tensor(t, uv, uv, Alu.mult)
            st[i]["v"], st[i]["d"], st[i]["t"] = uv, dd, t

        def p_h(i):
            b, k = divmod(i, 2)
            t, d = st[i]["t"], st[i]["d"]
            h = psh.tile([P, H], dt, tag="h", name=f"h{i}")
            eye = wt2[:, BPC * P:(BPC + 1) * P]
            nI = wt2[:, b * P:(b + 1) * P]
            for q in range(2):
                lo = PIECE * q
                nc.tensor.matmul(h[:, lo:lo + PIECE], eye, t[:, lo:lo + PIECE],
                                 start=True, stop=False)
            for q in range(2):
                lo = PIECE * q
                nc.tensor.matmul(h[:, lo:lo + PIECE], nI, d[:, lo:lo + PIECE],
                                 start=False, stop=True)
            st[i]["h"] = h

        def p_exp(i):
            g = st[i]["v"]  # overwrite v (dead)
            nc.scalar.activation(g, st[i]["h"], Af.Exp, scale=-1.0)
            st[i]["g"] = g

        def p_mul(i):
            b, k = divmod(i, 2)
            g = st[i]["g"]
            o = opool.tile([P, C, H], dh, tag="o", name=f"o{i}")
            grep = bass.AP(g.tensor, g.offset,
                           [[g.ap[0][0], P], [0, C], [1, H]])
            nc.vector.tensor_tensor(o, grep, s[b][:, :, k * H:(k + 1) * H],
                                    Alu.mult)
            nc.gpsimd.dma_start(
                bass.AP(out, b * C * P * U + k * H,
                        [[U, P], [P * U, C], [1, H]]), o)

        # ---- software-pipelined emission over 8 half-batch units ----
        for i in range(NU):
            p_in(i)
        nc.sync.dma_start(pc, pcols.ap())
        nc.sync.dma_start(wt, wts.ap())
        nc.sync.dma_start(wt2, wts2.ap())

        for i in range(NU + 4):
            if i < NU:
                p_sq(i)
                p_ee(i)
            if 1 <= i + 0 and i - 1 < NU and i >= 1:
                p_mm(i - 1)
            if i >= 3 and i - 3 < NU:
                p_exp(i - 3)
            if i >= 2 and i - 2 < NU:
                p_ln(i - 2)
            if i >= 4 and i - 4 < NU:
                p_mul(i - 4)
            if i >= 2 and i - 2 < NU:
                p_uvdt(i - 2)
                p_h(i - 2)

    nc.compile()
    return nc


def _get_program(M):
    key = ("nc", M)
    if key not in _cache:
        _cache[key] = _build_program(M)
    return _cache[key]


def _run(inputs, trace=False):
    from concourse.bass_utils import run_bass_kernel_spmd

    sig16 = np.asarray(inputs["input_signals"], np.float32).astype(np.float16)
    # layout B: [N, C, P, U] with [n,c,p,u] = sig[n,c,128*u+p]
    sB = np.ascontiguousarray(sig16.reshape(N, C, U, P).swapaxes(2, 3))
    pv, alpha, negc1, M = _host_params(
        np.asarray(inputs["z_alpha_pre"], np.float32),
        np.asarray(inputs["log_threshold"], np.float32),
        np.asarray(inputs["log_ratio"], np.float32),
        np.asarray(inputs["log_knee"], np.float32),
    )
    wts_all = _host_weights(alpha, M)
    nc = _get_program(M)

    eye = np.eye(P, dtype=np.float16)
    in_maps = []
    for cid in range(NCORES):
        bsl = slice(cid * BPC, (cid + 1) * BPC)
        core_sig = np.ascontiguousarray(
            sB[bsl].transpose(1, 2, 0, 3).reshape(C, P, BPC * U))
        cols = np.ascontiguousarray(
            np.tile(pv[bsl].reshape(1, BPC * NP), (P, 1)))
        wcore = np.ascontiguousarray(
            wts_all[bsl].transpose(2, 0, 1, 3).reshape(P, BPC * M * P))
        w2 = np.concatenate(
            [np.concatenate([eye * np.float16(negc1[n]) for n in
                             range(cid * BPC, (cid + 1) * BPC)], axis=1),
             eye], axis=1)
        in_maps.append({"sigB": core_sig, "pcols": cols, "wts": wcore,
                        "wts2": np.ascontiguousarray(w2)})

    res = run_bass_kernel_spmd(
        nc, in_maps, core_ids=list(range(NCORES)), trace=trace,
    )
    outp = np.empty((N, C, L), dtype=np.float32)
    for cid in range(NCORES):
        o = res.results[cid]["out"]  # [BPC, C, P, U] fp16
        outp[cid * BPC:(cid + 1) * BPC] = (
            o.transpose(0, 1, 3, 2).reshape(BPC, C, L).astype(np.float32))
    return outp, res


def kernel(**inputs) -> np.ndarray:
    out, _ = _run(inputs, trace=False)
    return out


# revision 9
# speedup vs baseline: 1.1671x; 1.0765x over previous
"""Trainium2 Bass kernel for nn_BaseCompressor2 (truncated one-pole IIR
compressor), v4: time-on-partitions layout, scan as a matmul FIR.

Layout B: per (batch, channel) the host pre-transposes the signal so that
SBUF partition i holds samples t with t mod 128 == i, free axis u = t div
128 (U = 2048 columns per batch).  All per-batch params are replicated
across the 128 partitions, so every elementwise op sees per-partition
scalars as usual.

The one-pole IIR (effective FIR window <= 128*M samples since alpha^k
underflows) becomes M matmul taps on the otherwise idle PE:
  y[:, j] = sum_m H_m @ E[:, j - m]        (f32 accum in PSUM)
with H_m[k, po] = alpha^(128 m + po - k) (H_0 lower-triangular).  Edge
columns j < m simply skip the tap (causal zero history).  No scan, no
carries, no cross-partition traffic.

Engine split per batch (U=2048 cols, all fp16):
  Act:  sq = Square(s) (both ch), x = Ln(escale*y+eps) from PSUM,
        g = Exp(-h) from PSUM
  DVE:  E = sq0+sq1, u=(x+uk)max0, v=(u min knee)*rta, d=(x+ukk)max0,
        t=v*v, out = g*s (channel-fused via stride-0 broadcast of g)
  PE :  M*4 FIR matmuls + h = eye@t + (negc1*eye)@d into PSUM
  DMA:  1 fat input DMA + 1 fat output DMA per batch (fp16 both ways)

Host does the (free) layout transposes and the final f32 cast.
"""

import numpy as np

N, C, L = 32, 2, 262144
NCORES = 8
BPC = N // NCORES          # batches per core
P = 128
U = L // P                 # 2048 free columns per batch
PIECE = 512                # psum bank width (f32)
NPIECE = U // PIECE
NP = 6                     # param columns per batch
ESCALE, UK, UKK, KNEE, RTA, NEGC1 = range(NP)

_cache = {}


def _host_params(z_alpha_pre, log_threshold, log_ratio, log_knee):
    z = z_alpha_pre.astype(np.float64).reshape(-1)
    thr = log_threshold.astype(np.float64).reshape(-1) - 6.0
    knee = np.exp(log_knee.astype(np.float64).reshape(-1))
    r001 = 1.0 + np.exp(log_ratio.astype(np.float64).reshape(-1)) + 0.001
    alpha = 1.0 / (1.0 + np.exp(-z))
    negc1 = 1.0 - 1.0 / r001
    vals = np.zeros((N, NP), dtype=np.float64)
    vals[:, ESCALE] = (1.0 - alpha) / 2.0
    vals[:, UK] = knee / 2.0 - thr
    vals[:, UKK] = -knee / 2.0 - thr
    vals[:, KNEE] = knee
    vals[:, RTA] = np.sqrt(negc1 / (2.0 * (knee + 0.001)))
    vals[:, NEGC1] = negc1
    # taps needed per batch: alpha^k < 6e-8 (fp16 subnormal floor) cut
    lna = np.log(alpha)
    kmax = np.ceil(16.7 / np.maximum(1e-9, -lna)).astype(np.int64)
    m_b = (kmax + 127) // 128 + 1
    M = int(min(16, max(m_b)))
    return vals.astype(np.float32), alpha, negc1, M


def _host_weights(alpha, M):
    """FIR tap matrices per batch: H[b, m][k, po] = a^(128m+po-k), masked."""
    po = np.arange(P)[None, :]
    k = np.arange(P)[:, None]
    out = np.zeros((N, M, P, P), dtype=np.float16)
    for n in range(N):
        lna = np.log(alpha[n])
        for m in range(M):
            e = (128 * m + po - k).astype(np.float64)
            h = np.exp(e * lna)
            h[e < 0] = 0.0
            h[h < 6e-8] = 0.0
            out[n, m] = h.astype(np.float16)
    return out


def _build_program(M):
    from contextlib import ExitStack

    import concourse.bacc as bacc
    import concourse.bass as bass
    import concourse.tile as tile
    from concourse import mybir

    dt = mybir.dt.float32
    dh = mybir.dt.float16
    Alu = mybir.AluOpType
    Af = mybir.ActivationFunctionType

    nc = bacc.Bacc(
        "TRN2", target_bir_lowering=False, debug=False,
        enable_asserts=False, num_devices=NCORES,
    )
    sigB = nc.dram_tensor("sigB", [C, P, BPC * U], dh, kind="ExternalInput")
    pcols = nc.dram_tensor("pcols", [P, BPC * NP], dt, kind="ExternalInput")
    wts = nc.dram_tensor("wts", [P, BPC * M * P], dh, kind="ExternalInput")
    # diag weights for h: cols [b*P:(b+1)*P] = negc1_b * I, then I
    wts2 = nc.dram_tensor("wts2", [P, (BPC + 1) * P], dh,
                          kind="ExternalInput")
    out = nc.dram_tensor("out", [BPC, C, P, U], dh, kind="ExternalOutput")

    H = U // 2             # half-batch columns

    with tile.TileContext(nc) as tc, ExitStack() as ctx:
        const = ctx.enter_context(tc.tile_pool(name="const", bufs=1))
        spool = ctx.enter_context(tc.tile_pool(name="sp", bufs=1))
        sqp = ctx.enter_context(tc.tile_pool(name="sq", bufs=3))
        epool = ctx.enter_context(tc.tile_pool(name="ep", bufs=2))
        wkp = ctx.enter_context(tc.tile_pool(name="wk", bufs=5))
        opool = ctx.enter_context(tc.tile_pool(name="op", bufs=3))
        psy = ctx.enter_context(tc.tile_pool(name="psy", bufs=2, space="PSUM"))
        psh = ctx.enter_context(tc.tile_pool(name="psh", bufs=2, space="PSUM"))

        pc = const.tile([P, BPC * NP], dt, tag="pc")
        wt = const.tile([P, BPC * M * P], dh, tag="wt")
        wt2 = const.tile([P, (BPC + 1) * P], dh, tag="wt2")
        epsc = const.tile([P, 1], dt, tag="epsc")

        def col(b, j):
            return pc[:, b * NP + j:b * NP + j + 1]

        # manual activation-table load: natural_log_exp_and_others (id 6)
        ld = mybir.InstLoadActFuncSet(
            name=nc.get_next_instruction_name(), act_func_set_id=6,
            ins=[], outs=[])
        ld.engine = mybir.EngineType.Activation
        nc.scalar.add_instruction(ld)
        nc.vector.memset(epsc, 1e-5)

        # ---- tiles / state ----
        s = [spool.tile([P, C, U], dh, tag=f"s{b}", name=f"s{b}")
             for b in range(BPC)]
        E = [None] * BPC
        NU = 2 * BPC       # half-batch pipeline units
        st = [dict() for _ in range(NU)]

        def p_in(i):
            b, k = divmod(i, 2)
            nc.sync.dma_start(
                s[b][:, :, k * H:(k + 1) * H],
                bass.AP(sigB, b * U + k * H,
                        [[BPC * U, P], [P * BPC * U, C], [1, H]]))

        def p_sq(i):
            b, k = divmod(i, 2)
            sq = sqp.tile([P, C, H], dh, tag="sq", name=f"sq{i}")
            nc.scalar.activation(sq, s[b][:, :, k * H:(k + 1) * H], Af.Square)
            st[i]["sq"] = sq

        def p_ee(i):
            b, k = divmod(i, 2)
            sq = st[i]["sq"]
            if k == 0:
                E[b] = epool.tile([P, U], dh, tag="E", name=f"E{b}")
            nc.vector.tensor_tensor(E[b][:, k * H:(k + 1) * H],
                                    sq[:, 0, :], sq[:, 1, :], Alu.add)

        def p_mm(i):
            b, k = divmod(i, 2)
            ep = E[b]
            y = psy.tile([P, H], dt, tag="y", name=f"y{i}")
            for m in range(M):
                w = wt[:, (b * M + m) * P:(b * M + m + 1) * P]
                for qq in range(2):
                    q = 2 * k + qq
                    lo = PIECE * q
                    off = m if q == 0 else 0
                    nc.tensor.matmul(
                        y[:, PIECE * qq + off:PIECE * (qq + 1)],
                        w, ep[:, lo + off - m:lo + PIECE - m],
                        start=(m == 0), stop=(m == M - 1),
                        skip_group_check=True)
            st[i]["y"] = y

        def p_ln(i):
            b, k = divmod(i, 2)
            x = wkp.tile([P, H], dh, tag="x", name=f"x{i}")
            nc.scalar.activation(x, st[i]["y"], Af.Ln,
                                 scale=col(b, ESCALE), bias=epsc[:, 0:1])
            st[i]["x"] = x

        def p_uvdt(i):
            b, k = divmod(i, 2)
            x = st[i]["x"]
            uv = wkp.tile([P, H], dh, tag="uv", name=f"uv{i}")
            dd = wkp.tile([P, H], dh, tag="d", name=f"d{i}")
            nc.vector.tensor_scalar(uv, x, col(b, UK), 0.0, Alu.add, Alu.max)
            nc.vector.tensor_scalar(dd, x, col(b, UKK), 0.0, Alu.add, Alu.max)
            nc.vector.tensor_scalar(uv, uv, col(b, KNEE), col(b, RTA),
                                    Alu.min, Alu.mult)
            t = x  # overwrite x (dead)
            nc.vector.tensor_tensor(t, uv, uv, Alu.mult)
            st[i]["v"], st[i]["d"], st[i]["t"] = uv, dd, t

        def p_h(i):
            b, k = divmod(i, 2)
            t, d = st[i]["t"], st[i]["d"]
            h = psh.tile([P, H], dt, tag="h", name=f"h{i}")
            eye = wt2[:, BPC * P:(BPC + 1) * P]
            nI = wt2[:, b * P:(b + 1) * P]
            for q in range(2):
                lo = PIECE * q
                nc.tensor.matmul(h[:, lo:lo + PIECE], eye, t[:, lo:lo + PIECE],
                                 start=True, stop=False)
            for q in range(2):
                lo = PIECE * q
                nc.tensor.matmul(h[:, lo:lo + PIECE], nI, d[:, lo:lo + PIECE],
                                 start=False, stop=True)
            st[i]["h"] = h

        def p_exp(i):
            g = st[i]["v"]  # overwrite v (dead)
            nc.scalar.activation(g, st[i]["h"], Af.Exp, scale=-1.0)
            st[i]["g"] = g

        def p_mul(i):
            b, k = divmod(i, 2)
            g = st[i]["g"]
            o = opool.tile([P, C, H], dh, tag="o", name=f"o{i}")
            grep = bass.AP(g.tensor, g.offset,
                           [[g.ap[0][0], P], [0, C], [1, H]])
            nc.vector.tensor_tensor(o, grep, s[b][:, :, k * H:(k + 1) * H],
                                    Alu.mult)
            nc.sync.dma_start(
                bass.AP(out, b * C * P * U + k * H,
                        [[U, P], [P * U, C], [1, H]]), o)

        def p_wt(b):
            # per-batch FIR weights: wt[:, b*M*P : (b+1)*M*P]
            nc.sync.dma_start(
                wt[:, b * M * P:(b + 1) * M * P],
                bass.AP(wts, b * M * P, [[BPC * M * P, P], [1, M * P]]))

        # ---- software-pipelined emission over 8 half-batch units ----
        nc.sync.dma_start(pc, pcols.ap())
        nc.sync.dma_start(wt2, wts2.ap())
        p_in(0)
        p_wt(0)
        p_in(1)
        p_wt(1)
        p_in(2); p_in(3)
        p_wt(2); p_wt(3)
        for i in range(4, NU):
            p_in(i)

        for i in range(NU + 4):
            if i < NU:
                p_sq(i)
                p_ee(i)
            if 1 <= i + 0 and i - 1 < NU and i >= 1:
                p_mm(i - 1)
            if i >= 3 and i - 3 < NU:
                p_exp(i - 3)
            if i >= 2 and i - 2 < NU:
                p_ln(i - 2)
            if i >= 4 and i - 4 < NU:
                p_mul(i - 4)
            if i >= 2 and i - 2 < NU:
                p_uvdt(i - 2)
                p_h(i - 2)

    nc.compile()
    return nc


def _get_program(M):
    key = ("nc", M)
    if key not in _cache:
        _cache[key] = _build_program(M)
    return _cache[key]


def _run(inputs, trace=False):
    from concourse.bass_utils import run_bass_kernel_spmd

    sig16 = np.asarray(inputs["input_signals"], np.float32).astype(np.float16)
    # layout B: [N, C, P, U] with [n,c,p,u] = sig[n,c,128*u+p]
    sB = np.ascontiguousarray(sig16.reshape(N, C, U, P).swapaxes(2, 3))
    pv, alpha, negc1, M = _host_params(
        np.asarray(inputs["z_alpha_pre"], np.float32),
        np.asarray(inputs["log_threshold"], np.float32),
        np.asarray(inputs["log_ratio"], np.float32),
        np.asarray(inputs["log_knee"], np.float32),
    )
    wts_all = _host_weights(alpha, M)
    nc = _get_program(M)

    eye = np.eye(P, dtype=np.float16)
    in_maps = []
    for cid in range(NCORES):
        bsl = slice(cid * BPC, (cid + 1) * BPC)
        core_sig = np.ascontiguousarray(
            sB[bsl].transpose(1, 2, 0, 3).reshape(C, P, BPC * U))
        cols = np.ascontiguousarray(
            np.tile(pv[bsl].reshape(1, BPC * NP), (P, 1)))
        wcore = np.ascontiguousarray(
            wts_all[bsl].transpose(2, 0, 1, 3).reshape(P, BPC * M * P))
        w2 = np.concatenate(
            [np.concatenate([eye * np.float16(negc1[n]) for n in
                             range(cid * BPC, (cid + 1) * BPC)], axis=1),
             eye], axis=1)
        in_maps.append({"sigB": core_sig, "pcols": cols, "wts": wcore,
                        "wts2": np.ascontiguousarray(w2)})

    res = run_bass_kernel_spmd(
        nc, in_maps, core_ids=list(range(NCORES)), trace=trace,
    )
    outp = np.empty((N, C, L), dtype=np.float32)
    for cid in range(NCORES):
        o = res.results[cid]["out"]  # [BPC, C, P, U] fp16
        outp[cid * BPC:(cid + 1) * BPC] = (
            o.transpose(0, 1, 3, 2).reshape(BPC, C, L).astype(np.float32))
    return outp, res


def kernel(**inputs) -> np.ndarray:
    out, _ = _run(inputs, trace=False)
    return out
